# revision 20
# baseline (speedup 1.0000x reference)
"""Causal self-attention (B=4, T=2048, H=8, hd=128, D=1024) on 8 trn2 cores.

Sharding: core c handles batch b = c//2, head-group g = c%2 (heads 4g..4g+4).
Each core computes the qkv projection for its 4 heads (bf16), rms-norm + rope
on q/k, v = l0*v + l1*ve, causal attention, and a partial c_proj ([T, D], bf16)
over its head group.  Host sums the two head-group partials per batch.

v2 changes vs the fp32r baseline:
  - everything bf16 on the PE (same 1 cycle/row rate, half the SBUF/DMA bytes)
  - QT/KT produced by DMA XBAR transposes (frees PE + ACT/DVE copies)
  - softmax denominators accumulated on DVE/GPSIMD into a [128, T] tile, then
    one tiny ones-matmul per (head, window) instead of full-width ones matmuls
  - c_proj overlapped with attention: heads {0,1} projected as PE filler work
    during heads {2,3}; head {2,3} partials accumulate in-place, bf16 out
"""
import sys

sys.path.insert(0, "/opt/trn_rl_repo")

import numpy as np

import concourse.bass as bass
import concourse.mybir as mybir
import concourse.tile as tile
from concourse import bacc
from concourse.bass import ts
from concourse.bass_utils import run_bass_kernel_spmd
from concourse.masks import make_identity

F32 = mybir.dt.float32
BF16 = mybir.dt.bfloat16
MULT = mybir.AluOpType.mult
ADD = mybir.AluOpType.add
AF = mybir.ActivationFunctionType

# ---- problem constants (hardcoded per the contract) ----
B, T, D = 4, 2048, 1024
H, HD = 8, 128
HG = 4          # heads per group
EG = HG * HD    # 512 cols per head-group
ATTN_SCALE = 0.12
RMS_EPS = 1.1920929e-07
P = 128
NT = T // P     # 16 t-blocks
ND = D // P     # 8 d-chunks
NW = T // 512   # 4 query windows
S2 = ATTN_SCALE * ATTN_SCALE

_CACHED = {}


def _rope_tables():
    af = (1.0 / 1024.0) ** np.linspace(0.0, 1.0, HD // 4, dtype=np.float32)
    af = np.concatenate([af, np.zeros(HD // 4, dtype=np.float32)])
    t = np.arange(T, dtype=np.float32)
    theta = np.einsum("i,j->ij", t, af)  # [T, 64]
    cos, sin = np.cos(theta), np.sin(theta)
    # cc: [c|c] per head; ss: [s|-s] per head -> [T, 512]
    cc1 = np.concatenate([cos, cos], axis=1)            # [T,128]
    ss1 = np.concatenate([sin, -sin], axis=1)           # [T,128]
    import ml_dtypes
    cc = np.tile(cc1, (1, HG)).astype(ml_dtypes.bfloat16)
    ss = np.tile(ss1, (1, HG)).astype(ml_dtypes.bfloat16)
    return cc, ss


def _masks():
    # tri[tk, c] = 1 if c >= tk  (the causal edge band)
    import ml_dtypes
    tk = np.arange(P)[:, None]
    c = np.arange(P)[None, :]
    return (c >= tk).astype(ml_dtypes.bfloat16)


def build(cfg=None):
    cfg = cfg or {}
    warmup = cfg.get("warmup", 28)
    gp_j = cfg.get("gp_j", 12)       # dacc adds: j >= gp_j on GPSIMD, rest DVE
    nc = bacc.Bacc("TRN2", target_bir_lowering=False, debug=False)

    xT = nc.dram_tensor("xT", [D, T], BF16, kind="ExternalInput")
    wT = nc.dram_tensor("wT", [D, 3 * EG], BF16, kind="ExternalInput")
    ve = nc.dram_tensor("ve", [T, EG], BF16, kind="ExternalInput")
    cpT = nc.dram_tensor("cpT", [EG, D], BF16, kind="ExternalInput")
    cc_t = nc.dram_tensor("cc", [T, EG], BF16, kind="ExternalInput")
    ss_t = nc.dram_tensor("ss", [T, EG], BF16, kind="ExternalInput")
    mk_t = nc.dram_tensor("mk", [P, P], BF16, kind="ExternalInput")
    sel_t = nc.dram_tensor("selm", [4, NW * P], BF16, kind="ExternalInput")
    out = nc.dram_tensor("out", [T, D], BF16, kind="ExternalOutput")

    xTr = xT.rearrange("(c p) t -> c p t", p=P)       # [8, 128, 2048]
    wr = wT.rearrange("(c p) e -> c p e", p=P)        # [8, 128, 1536]
    cpr = cpT.rearrange("(c p) d -> c p d", p=P)      # [4, 128, 1024]
    ver = ve.rearrange("(i p) e -> i p e", p=P)       # [16, 128, 512]
    ccr = cc_t.rearrange("(i p) e -> i p e", p=P)
    ssr = ss_t.rearrange("(i p) e -> i p e", p=P)

    with tile.TileContext(nc) as tc:
        with (
            tc.tile_pool(name="persist", bufs=1) as pp,
            tc.tile_pool(name="consts", bufs=1) as cp,
        ):
            # persistent tensors
            QT = [pp.tile([P, T], BF16, tag=f"QT{h}", name=f"QT{h}") for h in range(HG)]
            KT = [pp.tile([P, T], BF16, tag=f"KT{h}", name=f"KT{h}") for h in range(HG)]
            V = [pp.tile([P, EG], BF16, tag=f"V{i}", name=f"V{i}") for i in range(NT)]
            Yt = [pp.tile([P, T], BF16, tag=f"Y{h}", name=f"Y{h}") for h in range(HG)]
            OC = [pp.tile([P, D], BF16, tag=f"OC{t}", name=f"OC{t}") for t in range(NT)]
            tri = cp.tile([P, P], BF16, tag="tri")
            SK = cp.tile([P, NT, HG], F32, tag="SK")
            ones_b = cp.tile([P, 1], BF16, tag="ones_b")
            bias_q = cp.tile([P, 1], F32, tag="bias_q")
            bias_k = cp.tile([P, 1], F32, tag="bias_k")
            em = cp.tile([P, 7], BF16, tag="em", name="em")
            selmb = cp.tile([4, NW * P], BF16, tag="selmb", name="selmb")
            identb = cp.tile([P, P], BF16, tag="identb")
            nc.vector.memset(bias_q[:], RMS_EPS / S2)
            nc.vector.memset(bias_k[:], float(RMS_EPS))
            nc.sync.dma_start(tri[:], mk_t[:, :])
            nc.vector.memset(ones_b[:], 1.0)
            nc.vector.memset(em[:], 0.0)
            nc.vector.memset(em[:, 3:4], 1.0)
            nc.sync.dma_start(selmb[:], sel_t[:, :])
            selb = [selmb[:, ts(w, P)] for w in range(NW)]
            ident = cp.tile([P, P], F32, tag="ident")
            make_identity(nc, ident[:])
            nc.scalar.copy(identb[:], ident[:])

            # c_proj weights (tiny; fetch early)
            cpt = [cp.tile([P, D], BF16, tag=f"cpt{e}", name=f"cpt{e}")
                   for e in range(HG)]

            # ---------------- Phase A: projections, rms+rope, transposes ---
            with (
                tc.tile_pool(name="wpool", bufs=1) as wp,
                tc.tile_pool(name="xpool", bufs=4) as xp,
                tc.tile_pool(name="qkte", bufs=2) as qp,
                tc.tile_pool(name="ropetmp", bufs=2) as rp,
                tc.tile_pool(name="finpool", bufs=3) as fp,
                tc.tile_pool(name="tabs", bufs=2) as tp,
                tc.tile_pool(name="pA", bufs=2, space="PSUM") as pA,
                tc.tile_pool(name="pT", bufs=2, space="PSUM") as pT,
            ):
                xtis = {}

                def fetch_x(i):
                    if i < NT:
                        xi = xp.tile([P, ND, P], BF16, tag="xt", name="xt")
                        nc.sync.dma_start(
                            xi[:], xTr[:, :, ts(i, P)].rearrange("c p t -> p c t"))
                        xtis[i] = xi

                wqkv = [wp.tile([P, 3 * EG], BF16, tag=f"w{c}", name=f"w{c}")
                        for c in range(ND)]
                for c in range(2):
                    nc.sync.dma_start(wqkv[c][:], wr[c])
                fetch_x(0)
                fetch_x(1)
                for c in range(2, ND):
                    nc.sync.dma_start(wqkv[c][:], wr[c])
                fetch_x(2)
                fetch_x(3)
                for e in range(HG):
                    nc.scalar.dma_start(cpt[e][:], cpr[e])

                pendA = None

                def emit_transposes(fin_q, fin_k, tsl):
                    for h in range(HG):
                        ptr = pT.tile([P, P], BF16, tag="ptr", name="ptr")
                        nc.tensor.transpose(ptr[:], fin_q[:, ts(h, HD)],
                                            identb[:])
                        nc.scalar.copy(QT[h][:, tsl], ptr[:])
                    for h in range(HG):
                        ptr = pT.tile([P, P], BF16, tag="ptr", name="ptr")
                        nc.tensor.transpose(ptr[:], fin_k[:, ts(h, HD)],
                                            identb[:])
                        nc.vector.tensor_copy(KT[h][:, tsl], ptr[:])

                if warmup:
                    wt = wp.tile([P, EG], BF16, tag="warmsrc", name="warmsrc")
                    nc.vector.memset(wt[:], 0.0)
                    for wi in range(warmup):
                        pw = pA.tile([P, EG], F32, tag="ps", name="warm",
                                     bufs=2)
                        nc.tensor.matmul(pw[0:1, :], ones_b[:],
                                         wt[:], start=True, stop=True)

                for i in range(NT):
                    tsl = ts(i, P)
                    xti = xtis.pop(i)

                    ps = pA.tile([P, 3 * EG], F32, tag="ps", bufs=2)
                    psq, psk, psv = (ps[:, 0:EG], ps[:, EG:2 * EG],
                                     ps[:, 2 * EG:3 * EG])
                    fetch_x(i + 4)
                    # q first: its (longer) elementwise chain starts while the
                    # k/v projections are still streaming on PE
                    for c in range(ND):
                        nc.tensor.matmul(psq, xti[:, c, :], wqkv[c][:, 0:EG],
                                         start=(c == 0), stop=(c == ND - 1))
                    for c in range(ND):
                        nc.tensor.matmul(psk, xti[:, c, :],
                                         wqkv[c][:, EG:2 * EG],
                                         start=(c == 0), stop=(c == ND - 1))
                    for c in range(ND):
                        nc.tensor.matmul(psv, xti[:, c, :],
                                         wqkv[c][:, 2 * EG:3 * EG],
                                         start=(c == 0), stop=(c == ND - 1))

                    # --- v = psv + ve_scaled (lambdas folded on host) ---
                    vet = tp.tile([P, EG], BF16, tag="vet")
                    nc.sync.dma_start(vet[:], ver[i])
                    nc.vector.tensor_tensor(V[i][:], psv, vet[:], op=ADD)

                    # --- rms sumsq straight from PSUM ---
                    sq_scr = rp.tile([P, 2 * EG], F32, tag="sq_scr")
                    nc.scalar.activation(sq_scr[:, 0:EG], psq, AF.Square)
                    nc.scalar.activation(sq_scr[:, EG:2 * EG], psk, AF.Square)
                    ssq = rp.tile([P, 8], F32, tag="ssq")
                    nc.vector.tensor_reduce(
                        ssq[:], sq_scr[:].rearrange("p (g e) -> p g e", e=HD),
                        op=ADD, axis=mybir.AxisListType.X)
                    # scales: q gets 0.12 folded in; k scale is folded into the
                    # phase-B exp (per-partition scale), so only store recip.
                    sc = rp.tile([P, 8], F32, tag="sc")
                    nc.scalar.activation(sc[:, 0:4], ssq[:, 0:4], AF.Sqrt,
                                         scale=1.0 / (HD * S2), bias=bias_q[:])
                    nc.scalar.activation(sc[:, 4:8], ssq[:, 4:8], AF.Sqrt,
                                         scale=1.0 / HD, bias=bias_k[:])
                    rsc = rp.tile([P, 4], F32, tag="rsc")
                    nc.vector.reciprocal(rsc[:], sc[:, 0:4])
                    nc.vector.reciprocal(SK[:, i, :], sc[:, 4:8])

                    # --- q prescale to bf16 (per-head per-partition scale) ---
                    qs = qp.tile([P, EG], BF16, tag="qs")
                    for h in range(HG):
                        nc.scalar.activation(qs[:, ts(h, HD)], psq[:, ts(h, HD)],
                                             AF.Copy, scale=rsc[:, h:h + 1])
                    # --- k to bf16 SBUF (gpsimd cannot read PSUM) ---
                    kte = qp.tile([P, EG], BF16, tag="kte")
                    nc.scalar.copy(kte[:], psk)

                    cct = tp.tile([P, EG], BF16, tag="cct")
                    sst = tp.tile([P, EG], BF16, tag="sst")
                    nc.sync.dma_start(cct[:], ccr[i])
                    nc.sync.dma_start(sst[:], ssr[i])
                    c4 = cct[:].rearrange("p (h s e) -> p h s e", h=HG, s=2)
                    s4 = sst[:].rearrange("p (h s e) -> p h s e", h=HG, s=2)

                    def rope_side(eng, src, tag):
                        x4 = src.rearrange("p (h s e) -> p h s e", h=HG, s=2)
                        t1 = rp.tile([P, HG, 2, 64], BF16, tag=f"t1_{tag}",
                                     name=f"t1_{tag}")
                        t2 = rp.tile([P, HG, 2, 64], BF16, tag=f"t2_{tag}",
                                     name=f"t2_{tag}")
                        eng.tensor_tensor(t1[:], x4, c4, op=MULT)
                        eng.tensor_tensor(t2[:, :, 0, :], x4[:, :, 1, :],
                                          s4[:, :, 0, :], op=MULT)
                        eng.tensor_tensor(t2[:, :, 1, :], x4[:, :, 0, :],
                                          s4[:, :, 1, :], op=MULT)
                        fin = fp.tile([P, EG], BF16, tag=f"fin_{tag}",
                                      name=f"fin_{tag}")
                        f4 = fin[:].rearrange("p (h s e) -> p h s e", h=HG, s=2)
                        eng.tensor_tensor(f4, t1[:], t2[:], op=ADD)
                        return fin

                    fin_q = rope_side(nc.vector, qs[:], "q")
                    fin_k = rope_side(nc.gpsimd, kte[:], "k")

                    # transposes of the PREVIOUS block go after this block's
                    # projections in the PE queue (hides the elementwise chain)
                    if pendA is not None:
                        emit_transposes(*pendA)
                    pendA = (fin_q, fin_k, tsl)
                emit_transposes(*pendA)

            # ---------------- Phase B: attention + c_proj (h-outer) ---------
            with (
                tc.tile_pool(name="ptpool", bufs=5) as ptp,
                tc.tile_pool(name="daccp", bufs=2) as dp,
                tc.tile_pool(name="rpool", bufs=2) as rpl,
            ):
              with (
                tc.tile_pool(name="pS", bufs=1, space="PSUM") as pS,
                tc.tile_pool(name="pY", bufs=1, space="PSUM") as pY,
                tc.tile_pool(name="pO", bufs=1, space="PSUM") as pO,
              ):
                # Software pipeline: scores+exp producers run a few consumer
                # slots ahead of the PV consumers; c_proj matmuls for finished
                # head pairs sit in a deferred queue popped as PE filler work.
                LAG = 3
                cons_q = []
                fill_q = []

                class _St:
                    pass

                def ensure_acc(st):
                    if st.ps_y is None:
                        st.ps_y = [pY.tile([P, 512], F32, tag=f"psy{w}",
                                           name=f"psy{w}")
                                   for w in range(NW)]

                def consume(st, j, pt):
                    ensure_acc(st)
                    h = st.h
                    for w in range(j // 4, NW):
                        lo = max(512 * w, P * j)
                        loc, po = lo - P * j, lo - 512 * w
                        width = 512 * (w + 1) - lo
                        nc.tensor.matmul(
                            st.ps_y[w][:, po:512], V[j][:, ts(h, HD)],
                            pt[:, loc:loc + width],
                            start=(j == 0), stop=(j == 4 * w + 3))

                def cproj_half(pair, tb, half):
                    # one PSUM bank: accumulate the pair's two heads, then
                    # copy (pair 0) or accumulate in-place (pair 1) into OC
                    po = pO.tile([P, 512], F32, tag="po", name="po")
                    base = tb * P
                    for k2, hh in enumerate((2 * pair, 2 * pair + 1)):
                        nc.tensor.matmul(po[:],
                                         Yt[hh][:, base:base + P],
                                         cpt[hh][:, ts(half, 512)],
                                         start=(k2 == 0), stop=(k2 == 1))
                    osl = OC[tb][:, ts(half, 512)]
                    if pair == 0:
                        nc.vector.tensor_copy(osl, po[:])
                    else:
                        nc.vector.tensor_tensor(osl, osl, po[:], op=ADD)
                        nc.sync.dma_start(out[base:base + P, ts(half, 512)],
                                          osl)

                for h in range(HG):
                    st = _St()
                    st.h, st.ps_y = h, None
                    dacc = dp.tile([P, T], BF16, tag="dacc", name="dacc")
                    for j in range(NT):
                        cols = T - P * j
                        # alternate the leading PSUM buffer per j so both
                        # chunk buffers stay in rotation
                        if cols > 1024:
                            seq = ("psA", "psB") if j % 2 == 0 else \
                                  ("psB", "psA")
                            chunks = []
                            off = 0
                            k = 0
                            while off < cols:
                                tag = seq[k % 2]
                                k += 1
                                size = min(1024 if tag == "psA" else 512,
                                           cols - off)
                                chunks.append(
                                    (off, size, tag,
                                     1024 if tag == "psA" else 512))
                                off += size
                        elif cols > 512:
                            chunks = [(0, cols, "psA", 1024)]
                        else:
                            tag = "psA" if j % 2 == 0 else "psB"
                            chunks = [(0, cols, tag,
                                       1024 if tag == "psA" else 512)]

                        # guaranteed-ready filler right before the
                        # (dependency-waiting) first chunk matmul
                        npop = 0
                        while fill_q and npop < 2:
                            fill_q.pop(0)()
                            npop += 1
                        pt = ptp.tile([P, T], BF16, tag="pt", name="pt")

                        def sc_chunk(off, csz, tag, shp, j=j, pt=pt, h=h):
                            ps = pS.tile([P, shp], F32, tag=tag, name=tag)
                            for s0 in range(0, csz, 512):
                                sw = min(512, csz - s0)
                                nc.tensor.matmul(
                                    ps[:, s0:s0 + sw], KT[h][:, ts(j, P)],
                                    QT[h][:, P * j + off + s0:
                                           P * j + off + s0 + sw],
                                    start=True, stop=True)
                            nc.scalar.activation(
                                pt[:, off:off + csz], ps[:, 0:csz], AF.Exp,
                                scale=SK[:, j, h:h + 1])

                        sc_chunk(*chunks[0])
                        # causal mask on the diagonal 128-col band
                        nc.gpsimd.tensor_tensor(pt[:, 0:P], pt[:, 0:P],
                                                tri[:], op=MULT)
                        if len(chunks) > 1:
                            sc_chunk(*chunks[1])
                        while len(cons_q) > LAG:
                            cons_q.pop(0)()
                        npop = 0
                        while fill_q and npop < 2:
                            fill_q.pop(0)()
                            npop += 1
                        for ch in chunks[2:]:
                            sc_chunk(*ch)
                        cons_q.append(
                            lambda st=st, j=j, pt=pt: consume(st, j, pt))
                        # denominator accumulate (post-mask pt)
                        if j == 0:
                            nc.vector.tensor_copy(dacc[:, 0:T], pt[:, 0:T])
                        else:
                            eng = nc.vector if j < gp_j else nc.gpsimd
                            eng.tensor_tensor(dacc[:, P * j:T],
                                              dacc[:, P * j:T],
                                              pt[:, 0:cols], op=ADD)

                    def finish_head(st=st, dacc=dacc):
                        # denominators: ones matmuls over the accumulated tile
                        psr = pS.tile([4, 512], F32, tag="psB", name="psr")
                        for w in range(NW):
                            nc.tensor.matmul(psr[:], em[:, 3 - w:7 - w],
                                             dacc[:, ts(w, 512)],
                                             start=(w == 0), stop=(w == NW - 1))
                        rro = rpl.tile([4, 512], F32, tag="rro", name="rro")
                        nc.vector.reciprocal_approx_fast(rro[:], psr[:])
                        st.rrow = rpl.tile([4, 512], BF16, tag="rrow",
                                           name="rrow")
                        nc.vector.tensor_copy(st.rrow[:], rro[:])
                    cons_q.append(finish_head)

                    def norm_w(st, w):
                        ps_b = pS.tile([P, 512], F32, tag="psA", name="ps_b")
                        nc.tensor.matmul(ps_b[:], selb[w], st.rrow[:],
                                         start=True, stop=True)
                        bb = rpl.tile([P, 512], F32, tag="bb", name="bb")
                        nc.scalar.copy(bb[:], ps_b[:])
                        nc.vector.tensor_tensor(Yt[st.h][:, ts(w, 512)],
                                                st.ps_y[w][:], bb[:], op=MULT)
                    for w in range(NW):
                        cons_q.append(lambda st=st, w=w: norm_w(st, w))

                    if h == 1:
                        # pair-0 c_proj becomes filler during heads 2,3
                        def queue_pair0():
                            for tb in range(NT):
                                for half in range(2):
                                    fill_q.append(
                                        lambda tb=tb, half=half:
                                        cproj_half(0, tb, half))
                        cons_q.append(queue_pair0)
                while cons_q:
                    cons_q.pop(0)()
                while fill_q:
                    fill_q.pop(0)()

              # ---- c_proj tail: pair 1 accumulates into OC, DMA out ----
              with tc.tile_pool(name="pT2", bufs=2, space="PSUM") as pT2:
                for tb in range(NT):
                    po = pT2.tile([P, D], F32, tag="po2", name="po2")
                    base = tb * P
                    for half in range(2):
                        for k2, hh in enumerate((2, 3)):
                            nc.tensor.matmul(
                                po[:, ts(half, 512)],
                                Yt[hh][:, base:base + P],
                                cpt[hh][:, ts(half, 512)],
                                start=(k2 == 0), stop=(k2 == 1))
                    nc.vector.tensor_tensor(OC[tb][:], OC[tb][:], po[:],
                                            op=ADD)
                    nc.sync.dma_start(out[base:base + P, :], OC[tb][:])
    nc.compile()
    return nc


def _get_nc():
    if "nc" not in _CACHED:
        _CACHED["nc"] = build()
    return _CACHED["nc"]


def _try_install_profile_shim():
    try:
        import contextlib
        import ctypes
        import types

        if "antenv.axon_hooks" in sys.modules:
            return
        so_path = "/opt/axon/libaxon_pjrt.so"
        lib = ctypes.CDLL(so_path)
        if not hasattr(lib, "axon_start_nrt_profile"):
            return
        lib.axon_start_nrt_profile.argtypes = [ctypes.POINTER(ctypes.c_int64),
                                               ctypes.c_size_t]
        lib.axon_start_nrt_profile.restype = ctypes.c_int64
        lib.axon_stop_nrt_profile.argtypes = [ctypes.c_char_p]
        lib.axon_stop_nrt_profile.restype = ctypes.c_int64

        @contextlib.contextmanager
        def _hook(output_dir, device_ids):
            import jax

            jax.devices()
            if device_ids:
                ids = (ctypes.c_int64 * len(device_ids))(*device_ids)
                rc = lib.axon_start_nrt_profile(ids, len(device_ids))
            else:
                rc = lib.axon_start_nrt_profile(None, 0)
            if rc != 0:
                raise RuntimeError(f"axon_start_nrt_profile rc={rc}")
            try:
                yield
            finally:
                lib.axon_stop_nrt_profile(str(output_dir).encode())

        mod = types.ModuleType("antenv.axon_hooks")
        mod.set_axon_ntff_profile_hook = lambda h: None
        mod.get_axon_ntff_profile_hook = lambda: _hook
        import antenv

        antenv.axon_hooks = mod
        sys.modules["antenv.axon_hooks"] = mod
    except Exception:
        pass


LAST_EXEC_TIME_NS = None


def _prepare_in_maps(x, ve, sa_lambdas, qkv_w, c_proj_weight):
    import ml_dtypes
    bf16 = ml_dtypes.bfloat16
    x = np.asarray(x, dtype=np.float32)
    ve = np.asarray(ve, dtype=np.float32)
    sa_lambdas = np.asarray(sa_lambdas, dtype=np.float32)
    qkv_w = np.asarray(qkv_w, dtype=np.float32)
    c_proj_weight = np.asarray(c_proj_weight, dtype=np.float32)

    cc, ss = _rope_tables()
    mk = _masks()
    l0, l1 = float(sa_lambdas[0]), float(sa_lambdas[1])
    selm = np.zeros((4, 4 * P), dtype=np.float32)
    for w in range(4):
        selm[w, w * P:(w + 1) * P] = 1.0
    selm = selm.astype(bf16)

    in_maps = []
    for c in range(8):
        b, g = c // 2, c % 2
        gs, ge = g * EG, (g + 1) * EG
        wq = qkv_w[0, gs:ge, :]           # [512, 1024]
        wk = qkv_w[1, gs:ge, :]
        wv = qkv_w[2, gs:ge, :] * l0      # fold lambda0 into the v projection
        in_maps.append({
            "xT": np.ascontiguousarray(x[b].T).astype(bf16),          # [D, T]
            "wT": np.ascontiguousarray(
                np.concatenate([wq, wk, wv], axis=0).T).astype(bf16),  # [D,1536]
            "ve": (np.ascontiguousarray(
                ve[b].reshape(T, H, HD)[:, g * HG:(g + 1) * HG, :]
                .reshape(T, EG)) * l1).astype(bf16),                  # [T, 512]
            "cpT": np.ascontiguousarray(
                c_proj_weight[:, gs:ge].T).astype(bf16),              # [512, D]
            "cc": cc, "ss": ss, "mk": mk, "selm": selm,
        })
    return in_maps


def kernel(x, ve, sa_lambdas, qkv_w, c_proj_weight):
    global LAST_EXEC_TIME_NS
    in_maps = _prepare_in_maps(x, ve, sa_lambdas, qkv_w, c_proj_weight)
    _try_install_profile_shim()
    nc = _get_nc()
    res = run_bass_kernel_spmd(nc, in_maps, core_ids=list(range(8)), trace=True)
    LAST_EXEC_TIME_NS = res.exec_time_ns

    outs = [np.asarray(res.results[c]["out"], dtype=np.float32)
            for c in range(8)]
    full = np.stack([outs[2 * b] + outs[2 * b + 1] for b in range(B)], axis=0)
    return full.astype(np.float32)


# revision 28
# speedup vs baseline: 1.0324x; 1.0324x over previous
"""Causal self-attention (B=4, T=2048, H=8, hd=128, D=1024) on 8 trn2 cores.

Sharding: core c handles batch b = c//2, head-group g = c%2 (heads 4g..4g+4).
Each core computes the qkv projection for its 4 heads (bf16), rms-norm + rope
on q/k, v = l0*v + l1*ve, causal attention, and a partial c_proj ([T, D], bf16)
over its head group.  Host sums the two head-group partials per batch.

v2 changes vs the fp32r baseline:
  - everything bf16 on the PE (same 1 cycle/row rate, half the SBUF/DMA bytes)
  - QT/KT produced by DMA XBAR transposes (frees PE + ACT/DVE copies)
  - softmax denominators accumulated on DVE/GPSIMD into a [128, T] tile, then
    one tiny ones-matmul per (head, window) instead of full-width ones matmuls
  - c_proj overlapped with attention: heads {0,1} projected as PE filler work
    during heads {2,3}; head {2,3} partials accumulate in-place, bf16 out
"""
import sys

sys.path.insert(0, "/opt/trn_rl_repo")

import numpy as np

import concourse.bass as bass
import concourse.mybir as mybir
import concourse.tile as tile
from concourse import bacc
from concourse.bass import ts
from concourse.bass_utils import run_bass_kernel_spmd
from concourse.masks import make_identity

F32 = mybir.dt.float32
BF16 = mybir.dt.bfloat16
MULT = mybir.AluOpType.mult
ADD = mybir.AluOpType.add
AF = mybir.ActivationFunctionType

# ---- problem constants (hardcoded per the contract) ----
B, T, D = 4, 2048, 1024
H, HD = 8, 128
HG = 4          # heads per group
EG = HG * HD    # 512 cols per head-group
ATTN_SCALE = 0.12
RMS_EPS = 1.1920929e-07
P = 128
NT = T // P     # 16 t-blocks
ND = D // P     # 8 d-chunks
NW = T // 512   # 4 query windows
S2 = ATTN_SCALE * ATTN_SCALE

_CACHED = {}


def _rope_tables():
    af = (1.0 / 1024.0) ** np.linspace(0.0, 1.0, HD // 4, dtype=np.float32)
    af = np.concatenate([af, np.zeros(HD // 4, dtype=np.float32)])
    t = np.arange(T, dtype=np.float32)
    theta = np.einsum("i,j->ij", t, af)  # [T, 64]
    cos, sin = np.cos(theta), np.sin(theta)
    # cc: [c|c] per head; ss: [s|-s] per head -> [T, 512]
    cc1 = np.concatenate([cos, cos], axis=1)            # [T,128]
    ss1 = np.concatenate([sin, -sin], axis=1)           # [T,128]
    import ml_dtypes
    cc = np.tile(cc1, (1, HG)).astype(ml_dtypes.bfloat16)
    ss = np.tile(ss1, (1, HG)).astype(ml_dtypes.bfloat16)
    return cc, ss


def _masks():
    # tri[tk, c] = 1 if c >= tk  (the causal edge band)
    import ml_dtypes
    tk = np.arange(P)[:, None]
    c = np.arange(P)[None, :]
    return (c >= tk).astype(ml_dtypes.bfloat16)


def build(cfg=None):
    cfg = cfg or {}
    warmup = cfg.get("warmup", 28)
    gp_j = cfg.get("gp_j", 12)       # dacc adds: j >= gp_j on GPSIMD, rest DVE
    nc = bacc.Bacc("TRN2", target_bir_lowering=False, debug=False)

    xT = nc.dram_tensor("xT", [D, T], BF16, kind="ExternalInput")
    wT = nc.dram_tensor("wT", [D, 3 * EG], BF16, kind="ExternalInput")
    ve = nc.dram_tensor("ve", [T, EG], BF16, kind="ExternalInput")
    cpT = nc.dram_tensor("cpT", [EG, D], BF16, kind="ExternalInput")
    cc_t = nc.dram_tensor("cc", [T, EG], BF16, kind="ExternalInput")
    ss_t = nc.dram_tensor("ss", [T, EG], BF16, kind="ExternalInput")
    mk_t = nc.dram_tensor("mk", [P, P], BF16, kind="ExternalInput")
    sel_t = nc.dram_tensor("selm", [4, NW * P], BF16, kind="ExternalInput")
    out = nc.dram_tensor("out", [T, D], BF16, kind="ExternalOutput")

    xTr = xT.rearrange("(c p) t -> c p t", p=P)       # [8, 128, 2048]
    wr = wT.rearrange("(c p) e -> c p e", p=P)        # [8, 128, 1536]
    cpr = cpT.rearrange("(c p) d -> c p d", p=P)      # [4, 128, 1024]
    ver = ve.rearrange("(i p) e -> i p e", p=P)       # [16, 128, 512]
    ccr = cc_t.rearrange("(i p) e -> i p e", p=P)
    ssr = ss_t.rearrange("(i p) e -> i p e", p=P)

    with tile.TileContext(nc) as tc:
        with (
            tc.tile_pool(name="persist", bufs=1) as pp,
            tc.tile_pool(name="consts", bufs=1) as cp,
        ):
            # persistent tensors
            QT = [pp.tile([P, T], BF16, tag=f"QT{h}", name=f"QT{h}") for h in range(HG)]
            KT = [pp.tile([P, T], BF16, tag=f"KT{h}", name=f"KT{h}") for h in range(HG)]
            V = [pp.tile([P, EG], BF16, tag=f"V{i}", name=f"V{i}") for i in range(NT)]
            Yt = [pp.tile([P, T], BF16, tag=f"Y{h}", name=f"Y{h}") for h in range(HG)]
            OC = [pp.tile([P, D], BF16, tag=f"OC{t}", name=f"OC{t}") for t in range(NT)]
            tri = cp.tile([P, P], BF16, tag="tri")
            SK = cp.tile([P, NT, HG], F32, tag="SK")
            ones_b = cp.tile([P, 1], BF16, tag="ones_b")
            bias_q = cp.tile([P, 1], F32, tag="bias_q")
            bias_k = cp.tile([P, 1], F32, tag="bias_k")
            em = cp.tile([P, 7], BF16, tag="em", name="em")
            selmb = cp.tile([4, NW * P], BF16, tag="selmb", name="selmb")
            identb = cp.tile([P, P], BF16, tag="identb")
            nc.vector.memset(bias_q[:], RMS_EPS / S2)
            nc.vector.memset(bias_k[:], float(RMS_EPS))
            nc.sync.dma_start(tri[:], mk_t[:, :])
            nc.vector.memset(ones_b[:], 1.0)
            nc.vector.memset(em[:], 0.0)
            nc.vector.memset(em[:, 3:4], 1.0)
            nc.sync.dma_start(selmb[:], sel_t[:, :])
            selb = [selmb[:, ts(w, P)] for w in range(NW)]
            ident = cp.tile([P, P], F32, tag="ident")
            make_identity(nc, ident[:])
            nc.scalar.copy(identb[:], ident[:])

            # c_proj weights (tiny; fetch early)
            cpt = [cp.tile([P, D], BF16, tag=f"cpt{e}", name=f"cpt{e}")
                   for e in range(HG)]

            # ---------------- Phase A: projections, rms+rope, transposes ---
            with (
                tc.tile_pool(name="wpool", bufs=1) as wp,
                tc.tile_pool(name="xpool", bufs=4) as xp,
                tc.tile_pool(name="qkte", bufs=2) as qp,
                tc.tile_pool(name="ropetmp", bufs=2) as rp,
                tc.tile_pool(name="finpool", bufs=3) as fp,
                tc.tile_pool(name="tabs", bufs=2) as tp,
                tc.tile_pool(name="pA", bufs=2, space="PSUM") as pA,
                tc.tile_pool(name="pT", bufs=2, space="PSUM") as pT,
            ):
                xtis = {}

                def fetch_x(i):
                    if i < NT:
                        xi = xp.tile([P, ND, P], BF16, tag="xt", name="xt")
                        nc.sync.dma_start(
                            xi[:], xTr[:, :, ts(i, P)].rearrange("c p t -> p c t"))
                        xtis[i] = xi

                wqkv = [wp.tile([P, 3 * EG], BF16, tag=f"w{c}", name=f"w{c}")
                        for c in range(ND)]
                for c in range(2):
                    nc.sync.dma_start(wqkv[c][:], wr[c])
                fetch_x(0)
                fetch_x(1)
                for c in range(2, ND):
                    nc.sync.dma_start(wqkv[c][:], wr[c])
                fetch_x(2)
                fetch_x(3)
                for e in range(HG):
                    nc.scalar.dma_start(cpt[e][:], cpr[e])

                pendA = None

                def emit_transposes(fin_q, fin_k, tsl):
                    for h in range(HG):
                        ptr = pT.tile([P, P], BF16, tag="ptr", name="ptr")
                        nc.tensor.transpose(ptr[:], fin_q[:, ts(h, HD)],
                                            identb[:])
                        nc.scalar.copy(QT[h][:, tsl], ptr[:])
                    for h in range(HG):
                        ptr = pT.tile([P, P], BF16, tag="ptr", name="ptr")
                        nc.tensor.transpose(ptr[:], fin_k[:, ts(h, HD)],
                                            identb[:])
                        nc.vector.tensor_copy(KT[h][:, tsl], ptr[:])

                if warmup:
                    wt = wp.tile([P, EG], BF16, tag="warmsrc", name="warmsrc")
                    nc.vector.memset(wt[:], 0.0)
                    for wi in range(warmup):
                        pw = pA.tile([P, EG], F32, tag="ps", name="warm",
                                     bufs=2)
                        nc.tensor.matmul(pw[0:1, :], ones_b[:],
                                         wt[:], start=True, stop=True)

                for i in range(NT):
                    tsl = ts(i, P)
                    xti = xtis.pop(i)

                    ps = pA.tile([P, 3 * EG], F32, tag="ps", bufs=2)
                    psq, psk, psv = (ps[:, 0:EG], ps[:, EG:2 * EG],
                                     ps[:, 2 * EG:3 * EG])
                    fetch_x(i + 4)
                    # q first: its (longer) elementwise chain starts while the
                    # k/v projections are still streaming on PE
                    for c in range(ND):
                        nc.tensor.matmul(psq, xti[:, c, :], wqkv[c][:, 0:EG],
                                         start=(c == 0), stop=(c == ND - 1))
                    for c in range(ND):
                        nc.tensor.matmul(psk, xti[:, c, :],
                                         wqkv[c][:, EG:2 * EG],
                                         start=(c == 0), stop=(c == ND - 1))
                    for c in range(ND):
                        nc.tensor.matmul(psv, xti[:, c, :],
                                         wqkv[c][:, 2 * EG:3 * EG],
                                         start=(c == 0), stop=(c == ND - 1))

                    vet = tp.tile([P, EG], BF16, tag="vet")
                    nc.sync.dma_start(vet[:], ver[i])

                    # --- q/k to bf16 SBUF early (feeds rope + sumsq) ---
                    qte = qp.tile([P, EG], BF16, tag="qte")
                    nc.scalar.copy(qte[:], psq)
                    kte = qp.tile([P, EG], BF16, tag="kte")
                    nc.scalar.copy(kte[:], psk)

                    # --- rms sumsq (q squares on gpsimd, k on ACT) ---
                    sq_scr = rp.tile([P, 2 * EG], F32, tag="sq_scr")
                    nc.gpsimd.tensor_tensor(sq_scr[:, 0:EG], qte[:], qte[:],
                                            op=MULT)
                    nc.scalar.activation(sq_scr[:, EG:2 * EG], psk, AF.Square)
                    ssq = rp.tile([P, 8], F32, tag="ssq")
                    nc.vector.tensor_reduce(
                        ssq[:], sq_scr[:].rearrange("p (g e) -> p g e", e=HD),
                        op=ADD, axis=mybir.AxisListType.X)
                    # scales: q gets 0.12 folded in; k scale is folded into the
                    # phase-B exp (per-partition scale), so only store recip.
                    sc = rp.tile([P, 8], F32, tag="sc")
                    nc.scalar.activation(sc[:, 0:4], ssq[:, 0:4], AF.Sqrt,
                                         scale=1.0 / (HD * S2), bias=bias_q[:])
                    nc.scalar.activation(sc[:, 4:8], ssq[:, 4:8], AF.Sqrt,
                                         scale=1.0 / HD, bias=bias_k[:])
                    rsc = rp.tile([P, 4], F32, tag="rsc")
                    nc.vector.reciprocal(rsc[:], sc[:, 0:4])
                    nc.vector.reciprocal(SK[:, i, :], sc[:, 4:8])

                    cct = tp.tile([P, EG], BF16, tag="cct")
                    sst = tp.tile([P, EG], BF16, tag="sst")
                    nc.sync.dma_start(cct[:], ccr[i])
                    nc.sync.dma_start(sst[:], ssr[i])
                    c4 = cct[:].rearrange("p (h s e) -> p h s e", h=HG, s=2)
                    s4 = sst[:].rearrange("p (h s e) -> p h s e", h=HG, s=2)

                    def rope_side(eng, src, tag, dt):
                        x4 = src.rearrange("p (h s e) -> p h s e", h=HG, s=2)
                        t1 = rp.tile([P, HG, 2, 64], dt, tag=f"t1_{tag}",
                                     name=f"t1_{tag}")
                        t2 = rp.tile([P, HG, 2, 64], dt, tag=f"t2_{tag}",
                                     name=f"t2_{tag}")
                        eng.tensor_tensor(t1[:], x4, c4, op=MULT)
                        eng.tensor_tensor(t2[:, :, 0, :], x4[:, :, 1, :],
                                          s4[:, :, 0, :], op=MULT)
                        eng.tensor_tensor(t2[:, :, 1, :], x4[:, :, 0, :],
                                          s4[:, :, 1, :], op=MULT)
                        fin = fp.tile([P, EG], BF16, tag=f"fin_{tag}",
                                      name=f"fin_{tag}")
                        f4 = fin[:].rearrange("p (h s e) -> p h s e", h=HG, s=2)
                        eng.tensor_tensor(f4, t1[:], t2[:], op=ADD)
                        return fin

                    # q: rope on DVE in bf16 (2x mode), in parallel with the
                    # rms chain; per-head rms scale lands on the rope output
                    # right before the transposes (cheap 4x DVE).
                    rr_q = rope_side(nc.vector, qte[:], "q", BF16)
                    fin_q = qp.tile([P, EG], BF16, tag="fin_q2")
                    for h in range(HG):
                        nc.vector.tensor_scalar(
                            fin_q[:, ts(h, HD)], rr_q[:, ts(h, HD)],
                            rsc[:, h:h + 1], None, op0=MULT)
                    fin_k = rope_side(nc.gpsimd, kte[:], "k", BF16)

                    # v = psv + ve_scaled (lambdas folded on host); emitted
                    # last so it doesn't block the rope chain on the in-order
                    # DVE queue while psv is still accumulating
                    nc.vector.tensor_tensor(V[i][:], psv, vet[:], op=ADD)

                    # transposes of the PREVIOUS block go after this block's
                    # projections in the PE queue (hides the elementwise chain)
                    if pendA is not None:
                        emit_transposes(*pendA)
                    pendA = (fin_q, fin_k, tsl)
                emit_transposes(*pendA)

            # ---------------- Phase B: attention + c_proj (h-outer) ---------
            with (
                tc.tile_pool(name="ptpool", bufs=5) as ptp,
                tc.tile_pool(name="daccp", bufs=2) as dp,
                tc.tile_pool(name="rpool", bufs=2) as rpl,
            ):
              with (
                tc.tile_pool(name="pS", bufs=1, space="PSUM") as pS,
                tc.tile_pool(name="pY", bufs=1, space="PSUM") as pY,
                tc.tile_pool(name="pO", bufs=1, space="PSUM") as pO,
              ):
                # Software pipeline: scores+exp producers run a few consumer
                # slots ahead of the PV consumers; c_proj matmuls for finished
                # head pairs sit in a deferred queue popped as PE filler work.
                LAG = 3
                cons_q = []
                fill_q = []

                class _St:
                    pass

                def ensure_acc(st):
                    if st.ps_y is None:
                        st.ps_y = [pY.tile([P, 512], F32, tag=f"psy{w}",
                                           name=f"psy{w}")
                                   for w in range(NW)]

                def consume(st, j, pt):
                    ensure_acc(st)
                    h = st.h
                    for w in range(j // 4, NW):
                        lo = max(512 * w, P * j)
                        loc, po = lo - P * j, lo - 512 * w
                        width = 512 * (w + 1) - lo
                        nc.tensor.matmul(
                            st.ps_y[w][:, po:512], V[j][:, ts(h, HD)],
                            pt[:, loc:loc + width],
                            start=(j == 0), stop=(j == 4 * w + 3))

                def cproj_half(pair, tb, half):
                    # one PSUM bank: accumulate the pair's two heads, then
                    # copy (pair 0) or accumulate in-place (pair 1) into OC
                    po = pO.tile([P, 512], F32, tag="po", name="po")
                    base = tb * P
                    for k2, hh in enumerate((2 * pair, 2 * pair + 1)):
                        nc.tensor.matmul(po[:],
                                         Yt[hh][:, base:base + P],
                                         cpt[hh][:, ts(half, 512)],
                                         start=(k2 == 0), stop=(k2 == 1))
                    osl = OC[tb][:, ts(half, 512)]
                    if pair == 0:
                        nc.vector.tensor_copy(osl, po[:])
                    else:
                        nc.vector.tensor_tensor(osl, osl, po[:], op=ADD)
                        nc.sync.dma_start(out[base:base + P, ts(half, 512)],
                                          osl)

                for h in range(HG):
                    st = _St()
                    st.h, st.ps_y = h, None
                    dacc = dp.tile([P, T], BF16, tag="dacc", name="dacc")
                    for j in range(NT):
                        cols = T - P * j
                        # alternate the leading PSUM buffer per j so both
                        # chunk buffers stay in rotation
                        if cols > 1024:
                            seq = ("psA", "psB") if j % 2 == 0 else \
                                  ("psB", "psA")
                            chunks = []
                            off = 0
                            k = 0
                            while off < cols:
                                tag = seq[k % 2]
                                k += 1
                                size = min(1024 if tag == "psA" else 512,
                                           cols - off)
                                chunks.append(
                                    (off, size, tag,
                                     1024 if tag == "psA" else 512))
                                off += size
                        elif cols > 512:
                            chunks = [(0, cols, "psA", 1024)]
                        else:
                            tag = "psA" if j % 2 == 0 else "psB"
                            chunks = [(0, cols, tag,
                                       1024 if tag == "psA" else 512)]

                        # guaranteed-ready filler right before the
                        # (dependency-waiting) first chunk matmul
                        if fill_q:
                            fill_q.pop(0)()
                        pt = ptp.tile([P, T], BF16, tag="pt", name="pt")

                        def sc_chunk(off, csz, tag, shp, j=j, pt=pt, h=h):
                            ps = pS.tile([P, shp], F32, tag=tag, name=tag)
                            for s0 in range(0, csz, 512):
                                sw = min(512, csz - s0)
                                nc.tensor.matmul(
                                    ps[:, s0:s0 + sw], KT[h][:, ts(j, P)],
                                    QT[h][:, P * j + off + s0:
                                           P * j + off + s0 + sw],
                                    start=True, stop=True)
                            nc.scalar.activation(
                                pt[:, off:off + csz], ps[:, 0:csz], AF.Exp,
                                scale=SK[:, j, h:h + 1])

                        sc_chunk(*chunks[0])
                        # causal mask on the diagonal 128-col band
                        nc.gpsimd.tensor_tensor(pt[:, 0:P], pt[:, 0:P],
                                                tri[:], op=MULT)
                        if len(chunks) > 1:
                            sc_chunk(*chunks[1])
                        while len(cons_q) > LAG:
                            cons_q.pop(0)()
                        if fill_q:
                            fill_q.pop(0)()
                        for ch in chunks[2:]:
                            sc_chunk(*ch)
                        cons_q.append(
                            lambda st=st, j=j, pt=pt: consume(st, j, pt))
                        # denominator accumulate (post-mask pt)
                        if j == 0:
                            nc.vector.tensor_copy(dacc[:, 0:T], pt[:, 0:T])
                        else:
                            eng = nc.vector if j < gp_j else nc.gpsimd
                            eng.tensor_tensor(dacc[:, P * j:T],
                                              dacc[:, P * j:T],
                                              pt[:, 0:cols], op=ADD)

                    def finish_head(st=st, dacc=dacc):
                        # denominators: ones matmuls over the accumulated tile
                        psr = pS.tile([4, 512], F32, tag="psB", name="psr")
                        for w in range(NW):
                            nc.tensor.matmul(psr[:], em[:, 3 - w:7 - w],
                                             dacc[:, ts(w, 512)],
                                             start=(w == 0), stop=(w == NW - 1))
                        rro = rpl.tile([4, 512], F32, tag="rro", name="rro")
                        nc.vector.reciprocal_approx_fast(rro[:], psr[:])
                        st.rrow = rpl.tile([4, 512], BF16, tag="rrow",
                                           name="rrow")
                        nc.vector.tensor_copy(st.rrow[:], rro[:])
                    cons_q.append(finish_head)

                    def norm_w(st, w):
                        ps_b = pS.tile([P, 512], F32, tag="psA", name="ps_b")
                        nc.tensor.matmul(ps_b[:], selb[w], st.rrow[:],
                                         start=True, stop=True)
                        bb = rpl.tile([P, 512], F32, tag="bb", name="bb")
                        nc.scalar.copy(bb[:], ps_b[:])
                        nc.vector.tensor_tensor(Yt[st.h][:, ts(w, 512)],
                                                st.ps_y[w][:], bb[:], op=MULT)
                    for w in range(NW):
                        cons_q.append(lambda st=st, w=w: norm_w(st, w))

                    if h == 1:
                        # pair-0 c_proj becomes filler during heads 2,3
                        def queue_pair0():
                            for tb in range(NT):
                                for half in range(2):
                                    fill_q.append(
                                        lambda tb=tb, half=half:
                                        cproj_half(0, tb, half))
                        cons_q.append(queue_pair0)
                while cons_q:
                    cons_q.pop(0)()
                while fill_q:
                    fill_q.pop(0)()

              # ---- c_proj tail: pair 1 accumulates into OC, DMA out ----
              with tc.tile_pool(name="pT2", bufs=2, space="PSUM") as pT2:
                for tb in range(NT):
                    po = pT2.tile([P, D], F32, tag="po2", name="po2")
                    base = tb * P
                    for half in range(2):
                        for k2, hh in enumerate((2, 3)):
                            nc.tensor.matmul(
                                po[:, ts(half, 512)],
                                Yt[hh][:, base:base + P],
                                cpt[hh][:, ts(half, 512)],
                                start=(k2 == 0), stop=(k2 == 1))
                    nc.vector.tensor_tensor(OC[tb][:], OC[tb][:], po[:],
                                            op=ADD)
                    nc.sync.dma_start(out[base:base + P, :], OC[tb][:])
    nc.compile()
    return nc


def _get_nc():
    if "nc" not in _CACHED:
        _CACHED["nc"] = build()
    return _CACHED["nc"]


def _try_install_profile_shim():
    try:
        import contextlib
        import ctypes
        import types

        if "antenv.axon_hooks" in sys.modules:
            return
        so_path = "/opt/axon/libaxon_pjrt.so"
        lib = ctypes.CDLL(so_path)
        if not hasattr(lib, "axon_start_nrt_profile"):
            return
        lib.axon_start_nrt_profile.argtypes = [ctypes.POINTER(ctypes.c_int64),
                                               ctypes.c_size_t]
        lib.axon_start_nrt_profile.restype = ctypes.c_int64
        lib.axon_stop_nrt_profile.argtypes = [ctypes.c_char_p]
        lib.axon_stop_nrt_profile.restype = ctypes.c_int64

        @contextlib.contextmanager
        def _hook(output_dir, device_ids):
            import jax

            jax.devices()
            if device_ids:
                ids = (ctypes.c_int64 * len(device_ids))(*device_ids)
                rc = lib.axon_start_nrt_profile(ids, len(device_ids))
            else:
                rc = lib.axon_start_nrt_profile(None, 0)
            if rc != 0:
                raise RuntimeError(f"axon_start_nrt_profile rc={rc}")
            try:
                yield
            finally:
                lib.axon_stop_nrt_profile(str(output_dir).encode())

        mod = types.ModuleType("antenv.axon_hooks")
        mod.set_axon_ntff_profile_hook = lambda h: None
        mod.get_axon_ntff_profile_hook = lambda: _hook
        import antenv

        antenv.axon_hooks = mod
        sys.modules["antenv.axon_hooks"] = mod
    except Exception:
        pass


LAST_EXEC_TIME_NS = None


def _prepare_in_maps(x, ve, sa_lambdas, qkv_w, c_proj_weight):
    import ml_dtypes
    bf16 = ml_dtypes.bfloat16
    x = np.asarray(x, dtype=np.float32)
    ve = np.asarray(ve, dtype=np.float32)
    sa_lambdas = np.asarray(sa_lambdas, dtype=np.float32)
    qkv_w = np.asarray(qkv_w, dtype=np.float32)
    c_proj_weight = np.asarray(c_proj_weight, dtype=np.float32)

    cc, ss = _rope_tables()
    mk = _masks()
    l0, l1 = float(sa_lambdas[0]), float(sa_lambdas[1])
    selm = np.zeros((4, 4 * P), dtype=np.float32)
    for w in range(4):
        selm[w, w * P:(w + 1) * P] = 1.0
    selm = selm.astype(bf16)

    in_maps = []
    for c in range(8):
        b, g = c // 2, c % 2
        gs, ge = g * EG, (g + 1) * EG
        wq = qkv_w[0, gs:ge, :]           # [512, 1024]
        wk = qkv_w[1, gs:ge, :]
        wv = qkv_w[2, gs:ge, :] * l0      # fold lambda0 into the v projection
        in_maps.append({
            "xT": np.ascontiguousarray(x[b].T).astype(bf16),          # [D, T]
            "wT": np.ascontiguousarray(
                np.concatenate([wq, wk, wv], axis=0).T).astype(bf16),  # [D,1536]
            "ve": (np.ascontiguousarray(
                ve[b].reshape(T, H, HD)[:, g * HG:(g + 1) * HG, :]
                .reshape(T, EG)) * l1).astype(bf16),                  # [T, 512]
            "cpT": np.ascontiguousarray(
                c_proj_weight[:, gs:ge].T).astype(bf16),              # [512, D]
            "cc": cc, "ss": ss, "mk": mk, "selm": selm,
        })
    return in_maps


def kernel(x, ve, sa_lambdas, qkv_w, c_proj_weight):
    global LAST_EXEC_TIME_NS
    in_maps = _prepare_in_maps(x, ve, sa_lambdas, qkv_w, c_proj_weight)
    _try_install_profile_shim()
    nc = _get_nc()
    res = run_bass_kernel_spmd(nc, in_maps, core_ids=list(range(8)), trace=True)
    LAST_EXEC_TIME_NS = res.exec_time_ns

    outs = [np.asarray(res.results[c]["out"], dtype=np.float32)
            for c in range(8)]
    full = np.stack([outs[2 * b] + outs[2 * b + 1] for b in range(B)], axis=0)
    return full.astype(np.float32)
